# revision 9
# baseline (speedup 1.0000x reference)
"""JointMLPDecoder TRN2 kernel: per-joint LayerNorm + MLP (D=512 -> 2048 -> 3).

Sharding: 24 joints split 3-per-core across 8 NeuronCores (expert-style).
Host packs x as x^T [J, D, B] in bf16 so each core streams [d, b] tiles.

v2 structure (all-bf16 matmuls, joint-level software pipeline):
  per joint j:  stats phase (8 batch chunks):
                   mean / E[x^2] over d via ones-matmul on PE (bf16)
                   var accumulated into a per-joint tile
                one sqrt (ACT) + reciprocal (DVE) over the whole joint
                   -> 2 ACT table loads per joint instead of 2 per pair
  gemms of joint j-1 overlap the stats of joint j, processed in blocks
  of 4 batch chunks:
                   normalize: xn = (x - mu) * rstd (2 DVE ops, bf16 out)
                   per mc-chunk: one LDWEIGHTS serves the 4 batch chunks
                   (4x fewer weight loads), gelu on ACT (PSUM -> bf16),
                   gemm2 matmuls col-tiled via tile_position: batch chunk
                   bc -> col group g=bc%4, so the 4 accumulation chains
                   run concurrently in disjoint 32-col strips of the PE
                   array and land at partitions {0,32,64,96} of one PSUM
                   bank (no cross-partition reduction needed)
LayerNorm affine (ln_g, ln_b) is folded into w1/b1 on the host.
"""

import numpy as np
import ml_dtypes
from contextlib import ExitStack

import concourse.bass as bass
import concourse.bacc as bacc
import concourse.tile as tile
from concourse import mybir
from concourse import bass_utils

F32 = mybir.dt.float32
BF16 = mybir.dt.bfloat16
AF = mybir.ActivationFunctionType
ALU = mybir.AluOpType

B = 4096
J = 24
D = 512
M = 2048
NCORES = 8
JPC = J // NCORES          # 3 joints per core
BCH = 512                  # batch chunk (matmul N)
NBC = B // BCH             # 8
NDC = D // 128             # 4 contraction chunks for gemm1
NMC = M // 128             # 16 contraction chunks for gemm2
NGRP = 4                   # gemm2 col-tile groups (tile_position)
EPS = 1e-5
RD = 1.0 / D

_CACHE: dict = {}


def _bcast_dc(ap, n):
    """View a [128, BCH] AP as [128, n, BCH] with stride-0 middle dim."""
    new_ap = [list(ap.ap[0]), [0, n], list(ap.ap[-1])]
    return bass.AP(tensor=ap.tensor, offset=ap.offset, ap=new_ap)


def build_body(nc, tc, ctx, jpc=JPC, nbc=NBC, gelu=True):
    xT = nc.dram_tensor("xT", [jpc, D, B], BF16, kind="ExternalInput").ap()
    w1 = nc.dram_tensor("w1", [jpc, D, M], BF16, kind="ExternalInput").ap()
    b1 = nc.dram_tensor("b1", [jpc, 128, NMC], F32, kind="ExternalInput").ap()
    w2 = nc.dram_tensor("w2", [jpc, 128, NMC, 3], BF16, kind="ExternalInput").ap()
    b2 = nc.dram_tensor("b2", [jpc, 128, 1], F32, kind="ExternalInput").ap()
    ones = nc.dram_tensor("ones", [128, 128], BF16, kind="ExternalInput").ap()
    yT = nc.dram_tensor("yT", [jpc, 3, B], F32, kind="ExternalOutput").ap()

    consts = ctx.enter_context(tc.tile_pool(name="consts", bufs=1))
    wpool = ctx.enter_context(tc.tile_pool(name="wpool", bufs=2))
    xtpool = ctx.enter_context(tc.tile_pool(name="xtpool", bufs=2 * nbc))
    xsqpool = ctx.enter_context(tc.tile_pool(name="xsqpool", bufs=2))
    stpool = ctx.enter_context(tc.tile_pool(name="stpool", bufs=2))
    mspool = ctx.enter_context(tc.tile_pool(name="mspool", bufs=2))
    xnpool = ctx.enter_context(tc.tile_pool(name="xnpool", bufs=2))
    hpool = ctx.enter_context(tc.tile_pool(name="hpool", bufs=2))
    opool = ctx.enter_context(tc.tile_pool(name="opool", bufs=2))
    ps_stats = ctx.enter_context(tc.tile_pool(name="ps_stats", bufs=1, space="PSUM"))
    ps_g1 = ctx.enter_context(tc.tile_pool(name="ps_g1", bufs=1, space="PSUM"))
    ps_g2 = ctx.enter_context(tc.tile_pool(name="ps_g2", bufs=2, space="PSUM"))

    ones_t = consts.tile([128, 128], BF16)
    nc.sync.dma_start(out=ones_t, in_=ones)
    eps_t = consts.tile([128, 1], F32)
    nc.vector.memset(eps_t, EPS)

    jw = {}   # per-joint weight tiles
    jst = {}  # per-joint stats tiles: (mu_all bf16, std_all bf16)
    jxt = {}  # per-(j, bc) x tiles

    def emit_stats(j, bc, var_all, mu_all):
        """Stats for (j, bc): x load, mu / E[x^2] matmuls, var into var_all."""
        if bc == 0:
            w1_t = [wpool.tile([128, M], BF16, name=f"w1_{dc}", tag=f"w1_{dc}")
                    for dc in range(NDC)]
            for dc in range(NDC):
                nc.sync.dma_start(out=w1_t[dc],
                                  in_=w1[j, dc * 128:(dc + 1) * 128, :])
            w2_t = wpool.tile([128, NMC, 3], BF16, name="w2_t", tag="w2_t")
            nc.sync.dma_start(out=w2_t, in_=w2[j])
            b1_t = wpool.tile([128, NMC], F32, name="b1_t", tag="b1_t")
            nc.sync.dma_start(out=b1_t, in_=b1[j])
            b2r_t = wpool.tile([128, 1], F32, name="b2r_t", tag="b2r_t")
            nc.sync.dma_start(out=b2r_t, in_=b2[j])
            jw[j] = (w1_t, w2_t, b1_t, b2r_t)

        bsl = slice(bc * BCH, (bc + 1) * BCH)
        xt = xtpool.tile([128, NDC, BCH], BF16, name="xt", tag="xt")
        nc.sync.dma_start(
            out=xt,
            in_=xT[j, :, bsl].rearrange("(dc p) b -> p dc b", p=128),
        )
        jxt[(j, bc)] = xt

        xsq = xsqpool.tile([128, NDC, BCH], BF16, name="xsq", tag="xsq")
        nc.vector.tensor_mul(xsq.rearrange("p dc b -> p (dc b)"),
                             xt.rearrange("p dc b -> p (dc b)"),
                             xt.rearrange("p dc b -> p (dc b)"))

        ps_mu = ps_stats.tile([128, BCH], F32, name="ps_mu", tag="ps_mu")
        ps_ms = ps_stats.tile([128, BCH], F32, name="ps_ms", tag="ps_ms")
        for dc in range(NDC):
            nc.tensor.matmul(ps_mu, ones_t, xt[:, dc, :],
                             start=(dc == 0), stop=(dc == NDC - 1))
        for dc in range(NDC):
            nc.tensor.matmul(ps_ms, ones_t, xsq[:, dc, :],
                             start=(dc == 0), stop=(dc == NDC - 1))

        # mu = sum(x)/D (bf16); var = sum(x^2)/D - mu^2 (bf16)
        nc.vector.tensor_scalar_mul(mu_all[:, bc, :], ps_mu, RD)
        msq_t = mspool.tile([128, BCH], F32, name="msq_t", tag="msq_t")
        nc.vector.scalar_tensor_tensor(
            out=msq_t, in0=mu_all[:, bc, :], scalar=-1.0, in1=mu_all[:, bc, :],
            op0=ALU.mult, op1=ALU.mult)
        nc.vector.scalar_tensor_tensor(
            out=var_all[:, bc, :], in0=ps_ms, scalar=RD, in1=msq_t,
            op0=ALU.mult, op1=ALU.add)

    def emit_sqrt(j, var_all, mu_all):
        """One sqrt + reciprocal over the joint's full [128, nbc*BCH] var."""
        std_all = stpool.tile([128, nbc, BCH], BF16, name="std_all", tag="std_all")
        v_flat = var_all.rearrange("p i b -> p (i b)")
        s_flat = std_all.rearrange("p i b -> p (i b)")
        nc.scalar.activation(s_flat, v_flat, AF.Sqrt, bias=eps_t, scale=1.0)
        with nc.allow_low_precision(reason="bf16 rstd: 0.1% rms, tolerance 2e-2"):
            nc.vector.reciprocal(s_flat, s_flat)
        jst[j] = (mu_all, std_all)

    def emit_gemm_block(j, blk):
        """Gemms for a block of NGRP batch chunks of joint j.

        Loop order mc -> dc -> bc so one LDWEIGHTS serves NGRP matmuls;
        gemm2 chains for the NGRP chunks run in disjoint col groups.
        """
        w1_t, w2_t, b1_t, b2r_t = jw[j]
        mu_all, rstd_all = jst[j]
        bcs = [blk * NGRP + g for g in range(NGRP)]

        xns = []
        for g, bc in enumerate(bcs):
            xt = jxt.pop((j, bc))
            mu_b = _bcast_dc(mu_all[:, bc, :], NDC)
            rstd_b = _bcast_dc(rstd_all[:, bc, :], NDC)
            xn = xnpool.tile([128, NDC, BCH], BF16, name="xn", tag=f"xn{g}")
            nc.vector.tensor_sub(xn, xt, mu_b)
            nc.vector.tensor_mul(xn, xn, rstd_b)
            xns.append(xn)

        ps_y = ps_g2.tile([128, BCH], F32, name="ps_y", tag="ps_y")
        for mc in range(NMC):
            msl = slice(mc * 128, (mc + 1) * 128)
            ps_hs = [ps_g1.tile([128, BCH], F32, name="ps_h", tag=f"ps_h{g}")
                     for g in range(NGRP)]
            for dc in range(NDC):
                for g in range(NGRP):
                    nc.tensor.matmul(ps_hs[g], w1_t[dc][:, msl],
                                     xns[g][:, dc, :],
                                     start=(dc == 0), stop=(dc == NDC - 1))
            h_ts = []
            for g in range(NGRP):
                h_t = hpool.tile([128, BCH], BF16, name="h_t", tag=f"h_t{g}")
                nc.scalar.activation(h_t, ps_hs[g],
                                     AF.Gelu if gelu else AF.Identity,
                                     bias=b1_t[:, mc:mc + 1], scale=1.0)
                h_ts.append(h_t)
            for g in range(NGRP):
                nc.tensor.matmul(ps_y[32 * g:32 * g + 3, :], w2_t[:, mc, :],
                                 h_ts[g],
                                 start=(mc == 0), stop=(mc == NMC - 1),
                                 tile_position=(0, 32 * g),
                                 skip_group_check=True)

        y4 = opool.tile([128, BCH], F32, name="y4", tag="y4")
        for g, bc in enumerate(bcs):
            psl = slice(32 * g, 32 * g + 3)
            bsl = slice(bc * BCH, (bc + 1) * BCH)
            nc.vector.tensor_scalar_add(y4[psl, :], ps_y[psl, :], b2r_t[psl, :])
            nc.sync.dma_start(out=yT[j, :, bsl], in_=y4[psl, :])

    # joint-level pipeline: stats(j) + sqrt(j) ahead of gemms(j-1)
    nblk = nbc // NGRP
    for j in range(jpc):
        var_all = mspool.tile([128, nbc, BCH], BF16, name="var_all", tag="var_all")
        mu_all = stpool.tile([128, nbc, BCH], BF16, name="mu_all", tag="mu_all")
        for bc in range(nbc):
            emit_stats(j, bc, var_all, mu_all)
        emit_sqrt(j, var_all, mu_all)
        if j >= 1:
            for blk in range(nblk):
                emit_gemm_block(j - 1, blk)
    for blk in range(nblk):
        emit_gemm_block(jpc - 1, blk)


def _build_nc(jpc=JPC, nbc=NBC, reps=1, gelu=True):
    nc = bacc.Bacc("TRN2", target_bir_lowering=False, debug=False, num_devices=NCORES)
    with tile.TileContext(nc) as tc, ExitStack() as ctx:
        if reps == 1:
            build_body(nc, tc, ctx, jpc, nbc, gelu)
        else:
            # timing variant: repeat the whole body in a hardware loop
            def body(_i, unroll=1):
                with ExitStack() as c2:
                    build_body(nc, tc, c2, jpc, nbc, gelu)
            with tc.For_i(0, reps, 1) as i:
                body(i)
    nc.compile()
    return nc


def _pack_inputs(x, ln_g, ln_b, w1, b1, w2, b2):
    x = np.asarray(x)
    w1 = np.asarray(w1, dtype=np.float32)
    b1 = np.asarray(b1, dtype=np.float32)
    w2 = np.asarray(w2, dtype=np.float32)
    b2 = np.asarray(b2, dtype=np.float32)
    ln_g = np.asarray(ln_g)
    ln_b = np.asarray(ln_b)

    # fold LN affine into w1/b1: (xn*g + b) @ w1 == xn @ (g*w1) + b @ w1
    if not (ln_b == 0.0).all():
        b1 = b1 + np.einsum("jd,jdm->jm", ln_b, w1)
    if not (ln_g == 1.0).all():
        w1 = ln_g[:, :, None] * w1

    bf16 = ml_dtypes.bfloat16
    xT = np.ascontiguousarray(x.transpose(1, 2, 0)).astype(bf16)   # [J, D, B]
    w1p = w1.astype(bf16)
    w2p = np.ascontiguousarray(
        w2.reshape(J, NMC, 128, 3).transpose(0, 2, 1, 3)).astype(bf16)
    b1p = np.ascontiguousarray(
        b1.reshape(J, NMC, 128).transpose(0, 2, 1))                # [J, 128, NMC]
    # b2 replicated to partitions {0, 32, 64, 96} (gemm2 col-tile groups)
    b2p = np.zeros((J, 128, 1), dtype=np.float32)
    for g in range(NGRP):
        b2p[:, 32 * g:32 * g + 3, 0] = b2
    ones = np.ones((128, 128), dtype=bf16)

    in_maps = []
    for c in range(NCORES):
        js = slice(c * JPC, (c + 1) * JPC)
        in_maps.append({
            "xT": xT[js],
            "w1": np.ascontiguousarray(w1p[js]),
            "b1": b1p[js],
            "w2": w2p[js],
            "b2": b2p[js],
            "ones": ones,
        })
    return in_maps


def kernel(x, ln_g, ln_b, w1, b1, w2, b2):
    if "nc" not in _CACHE:
        _CACHE["nc"] = _build_nc()
    nc = _CACHE["nc"]

    in_maps = _pack_inputs(x, ln_g, ln_b, w1, b1, w2, b2)
    res = bass_utils.run_bass_kernel_spmd(nc, in_maps, core_ids=list(range(NCORES)))

    # yT per core: [JPC, 3, B] -> y [B, 1, J, 3]
    yT = np.stack([res.results[c]["yT"] for c in range(NCORES)])  # [8, JPC, 3, B]
    y = yT.reshape(J, 3, B).transpose(2, 0, 1)[:, None, :, :]
    return np.ascontiguousarray(y.astype(np.float32))


# revision 10
# speedup vs baseline: 1.0423x; 1.0423x over previous
"""JointMLPDecoder TRN2 kernel: per-joint LayerNorm + MLP (D=512 -> 2048 -> 3).

Sharding: 24 joints split 3-per-core across 8 NeuronCores (expert-style).
Host packs x as x^T [J, D, B] in bf16 so each core streams [d, b] tiles.

v3 structure (all-bf16 matmuls, two phases):
  Phase 1 (stats, all joints): per (j, bc): mean / E[x^2] over d via
    ones-matmuls (PSUM banks rotate through the gemm pool's 4 banks so
    DVE drains overlap the next chunk's matmuls). After each joint's 8
    chunks: one ACT sqrt + DVE reciprocal over [128, 8*512]. All three
    sqrts run back-to-back on ACT -> exactly 2 ACT table loads in the
    whole kernel (sqrt set once, gelu set once) instead of 2 per pair.
  Phase 2 (gemms): per joint, blocks of 4 batch chunks. x is re-loaded
    (DMA is far from the bottleneck; saves 48KB/partition of SBUF).
    Loop mc -> dc -> bc so one LDWEIGHTS serves 4 matmuls. gemm2
    matmuls are col-tiled via tile_position: batch chunk g gets col
    group g, so the 4 accumulation chains run concurrently in disjoint
    32-col strips and land at partitions {0,32,64,96} of one PSUM bank
    (no cross-partition reduction). gemm2 for chunk mc is emitted after
    gemm1 of chunk mc+1 so it never waits on the trailing gelu.
LayerNorm affine (ln_g, ln_b) is folded into w1/b1 on the host.
"""

import numpy as np
import ml_dtypes
from contextlib import ExitStack

import concourse.bass as bass
import concourse.bacc as bacc
import concourse.tile as tile
from concourse import mybir
from concourse import bass_utils

F32 = mybir.dt.float32
BF16 = mybir.dt.bfloat16
AF = mybir.ActivationFunctionType
ALU = mybir.AluOpType

B = 4096
J = 24
D = 512
M = 2048
NCORES = 8
JPC = J // NCORES          # 3 joints per core
BCH = 512                  # batch chunk (matmul N)
NBC = B // BCH             # 8
NDC = D // 128             # 4 contraction chunks for gemm1
NMC = M // 128             # 16 contraction chunks for gemm2
NGRP = 4                   # batch chunks per gemm block = gemm2 col groups
EPS = 1e-5
RD = 1.0 / D

_CACHE: dict = {}


def _bcast_dc(ap, n):
    """View a [128, BCH] AP as [128, n, BCH] with stride-0 middle dim."""
    new_ap = [list(ap.ap[0]), [0, n], list(ap.ap[-1])]
    return bass.AP(tensor=ap.tensor, offset=ap.offset, ap=new_ap)


def build_body(nc, tc, ctx, jpc=JPC, nbc=NBC, gelu=True):
    xT = nc.dram_tensor("xT", [jpc, D, B], BF16, kind="ExternalInput").ap()
    w1 = nc.dram_tensor("w1", [jpc, D, M], BF16, kind="ExternalInput").ap()
    b1 = nc.dram_tensor("b1", [jpc, 128, NMC], F32, kind="ExternalInput").ap()
    w2 = nc.dram_tensor("w2", [jpc, 128, NMC, 3], BF16, kind="ExternalInput").ap()
    b2 = nc.dram_tensor("b2", [jpc, 128, 1], F32, kind="ExternalInput").ap()
    ones = nc.dram_tensor("ones", [128, 128], BF16, kind="ExternalInput").ap()
    yT = nc.dram_tensor("yT", [jpc, 3, B], F32, kind="ExternalOutput").ap()

    consts = ctx.enter_context(tc.tile_pool(name="consts", bufs=1))
    wpool = ctx.enter_context(tc.tile_pool(name="wpool", bufs=2))
    xtpool = ctx.enter_context(tc.tile_pool(name="xtpool", bufs=4))
    x2pool = ctx.enter_context(tc.tile_pool(name="x2pool", bufs=2))
    xsqpool = ctx.enter_context(tc.tile_pool(name="xsqpool", bufs=2))
    stpool = ctx.enter_context(tc.tile_pool(name="stpool", bufs=jpc))
    mspool = ctx.enter_context(tc.tile_pool(name="mspool", bufs=2))
    xnpool = ctx.enter_context(tc.tile_pool(name="xnpool", bufs=2))
    hpool = ctx.enter_context(tc.tile_pool(name="hpool", bufs=2))
    opool = ctx.enter_context(tc.tile_pool(name="opool", bufs=2))
    ps_g1 = ctx.enter_context(tc.tile_pool(name="ps_g1", bufs=1, space="PSUM"))
    ps_g2 = ctx.enter_context(tc.tile_pool(name="ps_g2", bufs=2, space="PSUM"))

    ones_t = consts.tile([128, 128], BF16)
    nc.sync.dma_start(out=ones_t, in_=ones)
    eps_t = consts.tile([128, 1], F32)
    nc.vector.memset(eps_t, EPS)

    jw = {}   # per-joint weight tiles
    jst = {}  # per-joint stats: (mu_all bf16, rstd_all bf16)

    def ps_bank(i):
        """Rotating [128, BCH] f32 PSUM tile from the 4 gemm1 bank tags."""
        return ps_g1.tile([128, BCH], F32, name="ps_b", tag=f"ps_h{i % 4}")

    def emit_stats(j, bc, var_all, mu_all):
        """Stats for (j, bc): x load, mu / E[x^2] matmuls, var into var_all."""
        bsl = slice(bc * BCH, (bc + 1) * BCH)
        xt = xtpool.tile([128, NDC, BCH], BF16, name="xt", tag="xt")
        nc.sync.dma_start(
            out=xt,
            in_=xT[j, :, bsl].rearrange("(dc p) b -> p dc b", p=128),
        )
        xsq = xsqpool.tile([128, NDC, BCH], BF16, name="xsq", tag="xsq")
        nc.vector.tensor_mul(xsq.rearrange("p dc b -> p (dc b)"),
                             xt.rearrange("p dc b -> p (dc b)"),
                             xt.rearrange("p dc b -> p (dc b)"))

        k = j * nbc + bc
        ps_mu = ps_bank(2 * k)
        ps_ms = ps_bank(2 * k + 1)
        for dc in range(NDC):
            nc.tensor.matmul(ps_mu, ones_t, xt[:, dc, :],
                             start=(dc == 0), stop=(dc == NDC - 1))
        for dc in range(NDC):
            nc.tensor.matmul(ps_ms, ones_t, xsq[:, dc, :],
                             start=(dc == 0), stop=(dc == NDC - 1))

        # mu = sum(x)/D (bf16); var = sum(x^2)/D - mu^2 (bf16)
        nc.vector.tensor_scalar_mul(mu_all[:, bc, :], ps_mu, RD)
        msq_t = mspool.tile([128, BCH], F32, name="msq_t", tag="msq_t")
        nc.vector.scalar_tensor_tensor(
            out=msq_t, in0=mu_all[:, bc, :], scalar=-1.0, in1=mu_all[:, bc, :],
            op0=ALU.mult, op1=ALU.mult)
        nc.vector.scalar_tensor_tensor(
            out=var_all[:, bc, :], in0=ps_ms, scalar=RD, in1=msq_t,
            op0=ALU.mult, op1=ALU.add)

    def emit_sqrt(j, var_all, mu_all):
        """One sqrt + reciprocal over the joint's full [128, nbc*BCH] var."""
        std_all = stpool.tile([128, nbc, BCH], BF16, name="std_all", tag="std_all")
        v_flat = var_all.rearrange("p i b -> p (i b)")
        s_flat = std_all.rearrange("p i b -> p (i b)")
        nc.scalar.activation(s_flat, v_flat, AF.Sqrt, bias=eps_t, scale=1.0)
        with nc.allow_low_precision(reason="bf16 rstd: 0.1% rms, tol 2e-2"):
            nc.vector.reciprocal(s_flat, s_flat)
        jst[j] = (mu_all, std_all)

    def emit_weights(j):
        w1_t = [wpool.tile([128, M], BF16, name=f"w1_{dc}", tag=f"w1_{dc}")
                for dc in range(NDC)]
        for dc in range(NDC):
            nc.sync.dma_start(out=w1_t[dc],
                              in_=w1[j, dc * 128:(dc + 1) * 128, :])
        w2_t = wpool.tile([128, NMC, 3], BF16, name="w2_t", tag="w2_t")
        nc.sync.dma_start(out=w2_t, in_=w2[j])
        b1_t = wpool.tile([128, NMC], F32, name="b1_t", tag="b1_t")
        nc.sync.dma_start(out=b1_t, in_=b1[j])
        b2r_t = wpool.tile([128, 1], F32, name="b2r_t", tag="b2r_t")
        nc.sync.dma_start(out=b2r_t, in_=b2[j])
        jw[j] = (w1_t, w2_t, b1_t, b2r_t)

    def emit_gemm_block(j, blk):
        """Gemms for a block of NGRP batch chunks of joint j.

        Loop order mc -> dc -> bc so one LDWEIGHTS serves NGRP matmuls;
        gemm2 for chunk mc is emitted after gemm1 of chunk mc+1 so it
        never waits on the trailing gelu (in-order PE).
        """
        w1_t, w2_t, b1_t, b2r_t = jw[j]
        mu_all, rstd_all = jst[j]
        bcs = [blk * NGRP + g for g in range(NGRP)]

        xns = []
        for g, bc in enumerate(bcs):
            bsl = slice(bc * BCH, (bc + 1) * BCH)
            xt2 = x2pool.tile([128, NDC, BCH], BF16, name="xt2", tag=f"xt2_{g}")
            nc.sync.dma_start(
                out=xt2,
                in_=xT[j, :, bsl].rearrange("(dc p) b -> p dc b", p=128),
            )
            mu_b = _bcast_dc(mu_all[:, bc, :], NDC)
            rstd_b = _bcast_dc(rstd_all[:, bc, :], NDC)
            xn = xnpool.tile([128, NDC, BCH], BF16, name="xn", tag=f"xn{g}")
            nc.vector.tensor_sub(xn, xt2, mu_b)
            nc.vector.tensor_mul(xn, xn, rstd_b)
            xns.append(xn)

        ps_y = ps_g2.tile([128, BCH], F32, name="ps_y", tag="ps_y")
        prev = None  # (h_ts, mc) whose gemm2 is pending
        for mc in range(NMC):
            msl = slice(mc * 128, (mc + 1) * 128)
            ps_hs = [ps_g1.tile([128, BCH], F32, name="ps_h", tag=f"ps_h{g}")
                     for g in range(NGRP)]
            for dc in range(NDC):
                for g in range(NGRP):
                    nc.tensor.matmul(ps_hs[g], w1_t[dc][:, msl],
                                     xns[g][:, dc, :],
                                     start=(dc == 0), stop=(dc == NDC - 1))
            if prev is not None:
                emit_g2(ps_y, w2_t, *prev)
            h_ts = []
            for g in range(NGRP):
                h_t = hpool.tile([128, BCH], BF16, name="h_t", tag=f"h_t{g}")
                nc.scalar.activation(h_t, ps_hs[g],
                                     AF.Gelu if gelu else AF.Identity,
                                     bias=b1_t[:, mc:mc + 1], scale=1.0)
                h_ts.append(h_t)
            prev = (h_ts, mc)
        emit_g2(ps_y, w2_t, *prev)

        y4 = opool.tile([128, BCH], F32, name="y4", tag="y4")
        for g, bc in enumerate(bcs):
            psl = slice(32 * g, 32 * g + 3)
            bsl = slice(bc * BCH, (bc + 1) * BCH)
            nc.vector.tensor_scalar_add(y4[psl, :], ps_y[psl, :], b2r_t[psl, :])
            nc.sync.dma_start(out=yT[j, :, bsl], in_=y4[psl, :])

    def emit_g2(ps_y, w2_t, h_ts, mc):
        for g in range(NGRP):
            nc.tensor.matmul(ps_y[32 * g:32 * g + 3, :], w2_t[:, mc, :],
                             h_ts[g],
                             start=(mc == 0), stop=(mc == NMC - 1),
                             tile_position=(0, 32 * g),
                             skip_group_check=True)

    # Phase 1: stats for all joints, sqrts back-to-back on ACT
    for j in range(jpc):
        var_all = mspool.tile([128, nbc, BCH], BF16, name="var_all", tag="var_all")
        mu_all = stpool.tile([128, nbc, BCH], BF16, name="mu_all", tag="mu_all")
        for bc in range(nbc):
            emit_stats(j, bc, var_all, mu_all)
        emit_sqrt(j, var_all, mu_all)
        if j == 0:
            emit_weights(0)

    # Phase 2: gemms
    nblk = nbc // NGRP
    for j in range(jpc):
        if j + 1 < jpc:
            emit_weights(j + 1)
        for blk in range(nblk):
            emit_gemm_block(j, blk)


def _build_nc(jpc=JPC, nbc=NBC, reps=1, gelu=True):
    nc = bacc.Bacc("TRN2", target_bir_lowering=False, debug=False, num_devices=NCORES)
    with tile.TileContext(nc) as tc, ExitStack() as ctx:
        if reps == 1:
            build_body(nc, tc, ctx, jpc, nbc, gelu)
        else:
            # timing variant: repeat the whole body in a hardware loop
            def body(_i, unroll=1):
                with ExitStack() as c2:
                    build_body(nc, tc, c2, jpc, nbc, gelu)
            with tc.For_i(0, reps, 1) as i:
                body(i)
    nc.compile()
    return nc


def _pack_inputs(x, ln_g, ln_b, w1, b1, w2, b2):
    x = np.asarray(x)
    w1 = np.asarray(w1, dtype=np.float32)
    b1 = np.asarray(b1, dtype=np.float32)
    w2 = np.asarray(w2, dtype=np.float32)
    b2 = np.asarray(b2, dtype=np.float32)
    ln_g = np.asarray(ln_g)
    ln_b = np.asarray(ln_b)

    # fold LN affine into w1/b1: (xn*g + b) @ w1 == xn @ (g*w1) + b @ w1
    if not (ln_b == 0.0).all():
        b1 = b1 + np.einsum("jd,jdm->jm", ln_b, w1)
    if not (ln_g == 1.0).all():
        w1 = ln_g[:, :, None] * w1

    bf16 = ml_dtypes.bfloat16
    xT = np.ascontiguousarray(x.transpose(1, 2, 0)).astype(bf16)   # [J, D, B]
    w1p = w1.astype(bf16)
    w2p = np.ascontiguousarray(
        w2.reshape(J, NMC, 128, 3).transpose(0, 2, 1, 3)).astype(bf16)
    b1p = np.ascontiguousarray(
        b1.reshape(J, NMC, 128).transpose(0, 2, 1))                # [J, 128, NMC]
    # b2 replicated to partitions {0, 32, 64, 96} (gemm2 col-tile groups)
    b2p = np.zeros((J, 128, 1), dtype=np.float32)
    for g in range(NGRP):
        b2p[:, 32 * g:32 * g + 3, 0] = b2
    ones = np.ones((128, 128), dtype=bf16)

    in_maps = []
    for c in range(NCORES):
        js = slice(c * JPC, (c + 1) * JPC)
        in_maps.append({
            "xT": xT[js],
            "w1": np.ascontiguousarray(w1p[js]),
            "b1": b1p[js],
            "w2": w2p[js],
            "b2": b2p[js],
            "ones": ones,
        })
    return in_maps


def kernel(x, ln_g, ln_b, w1, b1, w2, b2):
    if "nc" not in _CACHE:
        _CACHE["nc"] = _build_nc()
    nc = _CACHE["nc"]

    in_maps = _pack_inputs(x, ln_g, ln_b, w1, b1, w2, b2)
    res = bass_utils.run_bass_kernel_spmd(nc, in_maps, core_ids=list(range(NCORES)))

    # yT per core: [JPC, 3, B] -> y [B, 1, J, 3]
    yT = np.stack([res.results[c]["yT"] for c in range(NCORES)])  # [8, JPC, 3, B]
    y = yT.reshape(J, 3, B).transpose(2, 0, 1)[:, None, :, :]
    return np.ascontiguousarray(y.astype(np.float32))


# revision 38
# speedup vs baseline: 1.1797x; 1.1318x over previous
"""JointMLPDecoder TRN2 kernel: per-joint LayerNorm + MLP (D=512 -> 2048 -> 3).

Sharding: 24 joints split 3-per-core across 8 NeuronCores (expert-style).
Host packs x as x^T [J, D, B] in bf16 so each core streams [d, b] tiles.

v3 structure (all-bf16 matmuls, two phases):
  Phase 1 (stats, all joints): per (j, bc): mean / E[x^2] over d via
    ones-matmuls (PSUM banks rotate through the gemm pool's 4 banks so
    DVE drains overlap the next chunk's matmuls). After each joint's 8
    chunks: one ACT sqrt + DVE reciprocal over [128, 8*512]. All three
    sqrts run back-to-back on ACT -> exactly 2 ACT table loads in the
    whole kernel (sqrt set once, gelu set once) instead of 2 per pair.
  Phase 2 (gemms): per joint, blocks of 4 batch chunks. x is re-loaded
    (DMA is far from the bottleneck; saves 48KB/partition of SBUF).
    Loop mc -> dc -> bc so one LDWEIGHTS serves 4 matmuls. gemm2
    matmuls are col-tiled via tile_position: batch chunk g gets col
    group g, so the 4 accumulation chains run concurrently in disjoint
    32-col strips and land at partitions {0,32,64,96} of one PSUM bank
    (no cross-partition reduction). gemm2 for chunk mc is emitted after
    gemm1 of chunk mc+1 so it never waits on the trailing gelu.
LayerNorm affine (ln_g, ln_b) is folded into w1/b1 on the host.
"""

import numpy as np
import ml_dtypes
from contextlib import ExitStack

import concourse.bass as bass
import concourse.bacc as bacc
import concourse.tile as tile
from concourse import mybir
from concourse import bass_utils

F32 = mybir.dt.float32
BF16 = mybir.dt.bfloat16
AF = mybir.ActivationFunctionType
ALU = mybir.AluOpType

B = 4096
J = 24
D = 512
M = 2048
NCORES = 8
JPC = J // NCORES          # 3 joints per core
BCH = 512                  # batch chunk (matmul N)
NBC = B // BCH             # 8
NDC = D // 128             # 4 contraction chunks for gemm1
NMC = M // 128             # 16 contraction chunks for gemm2
NGRP = 4                   # batch chunks per gemm block = gemm2 col groups
EPS = 1e-5
RD = 1.0 / D

_CACHE: dict = {}


def _bcast_dc(ap, n):
    """View a [128, BCH] AP as [128, n, BCH] with stride-0 middle dim."""
    new_ap = [list(ap.ap[0]), [0, n], list(ap.ap[-1])]
    return bass.AP(tensor=ap.tensor, offset=ap.offset, ap=new_ap)


def build_body(nc, tc, ctx, jpc=JPC, nbc=NBC, gelu=True,
               serial_g2=False, g_outer=False,
               half_gelu=False, no_g2=False, no_norm=False,
               ngrp=NGRP, ps2x=False,
               half_g1=False, const_xn=False, no_phase1=False):
    xT = nc.dram_tensor("xT", [jpc, D, B], BF16, kind="ExternalInput").ap()
    w1 = nc.dram_tensor("w1", [jpc, D, M], BF16, kind="ExternalInput").ap()
    b1 = nc.dram_tensor("b1", [jpc, 128, NMC], F32, kind="ExternalInput").ap()
    w2 = nc.dram_tensor("w2", [jpc, 128, NMC, 3], BF16, kind="ExternalInput").ap()
    b2 = nc.dram_tensor("b2", [jpc, 128, 1], F32, kind="ExternalInput").ap()
    ones = nc.dram_tensor("ones", [128, 128], BF16, kind="ExternalInput").ap()
    yT = nc.dram_tensor("yT", [jpc, 3, B], F32, kind="ExternalOutput").ap()

    consts = ctx.enter_context(tc.tile_pool(name="consts", bufs=1))
    wpool = ctx.enter_context(tc.tile_pool(name="wpool", bufs=2))
    xtpool = ctx.enter_context(tc.tile_pool(name="xtpool", bufs=4))
    x2pool = ctx.enter_context(tc.tile_pool(name="x2pool", bufs=2))
    xsqpool = ctx.enter_context(tc.tile_pool(name="xsqpool", bufs=2))
    stpool = ctx.enter_context(tc.tile_pool(name="stpool", bufs=jpc))
    mspool = ctx.enter_context(tc.tile_pool(name="mspool", bufs=2))
    xnpool = ctx.enter_context(tc.tile_pool(name="xnpool", bufs=2))
    hpool = ctx.enter_context(tc.tile_pool(name="hpool", bufs=2))
    opool = ctx.enter_context(tc.tile_pool(name="opool", bufs=2))
    ps_g1 = ctx.enter_context(tc.tile_pool(name="ps_g1", bufs=1, space="PSUM"))
    ps_g2 = ctx.enter_context(tc.tile_pool(name="ps_g2", bufs=2, space="PSUM"))

    ones_t = consts.tile([128, 128], BF16)
    nc.sync.dma_start(out=ones_t, in_=ones)
    eps_t = consts.tile([128, 1], F32)
    nc.vector.memset(eps_t, EPS)

    jw = {}   # per-joint weight tiles
    jst = {}  # per-joint stats: (mu_all bf16, rstd_all bf16)

    cxn = None
    if const_xn:  # perf probe: constant gemm1 inputs (wrong results)
        cxn = [consts.tile([128, NDC, BCH], BF16, name=f"cxn{g}")
               for g in range(ngrp)]
        for g in range(ngrp):
            nc.vector.memset(cxn[g], 0.125)

    ps_h_bufs = 2 if (ps2x or ngrp == 2) else 1

    def ps_bank(i):
        """Rotating [128, BCH] f32 PSUM tile from the gemm1 bank tags."""
        return ps_g1.tile([128, BCH], F32, name="ps_b",
                          tag=f"ps_h{i % ngrp}", bufs=ps_h_bufs)

    def emit_stats(j, bc, var_all, mu_all):
        """Stats for (j, bc): x load, mu / E[x^2] matmuls, var into var_all."""
        bsl = slice(bc * BCH, (bc + 1) * BCH)
        xt = xtpool.tile([128, NDC, BCH], BF16, name="xt", tag="xt")
        nc.sync.dma_start(
            out=xt,
            in_=xT[j, :, bsl].rearrange("(dc p) b -> p dc b", p=128),
        )
        # x^2 on ACT (Square lives in every table set; ACT is idle in phase 1)
        xsq = xsqpool.tile([128, NDC, BCH], BF16, name="xsq", tag="xsq")
        nc.scalar.activation(xsq.rearrange("p dc b -> p (dc b)"),
                             xt.rearrange("p dc b -> p (dc b)"), AF.Square)

        k = j * nbc + bc
        ps_mu = ps_bank(2 * k)
        ps_ms = ps_bank(2 * k + 1)
        for dc in range(NDC):
            nc.tensor.matmul(ps_mu, ones_t, xt[:, dc, :],
                             start=(dc == 0), stop=(dc == NDC - 1))
        for dc in range(NDC):
            nc.tensor.matmul(ps_ms, ones_t, xsq[:, dc, :],
                             start=(dc == 0), stop=(dc == NDC - 1))

        # ones carry 1/D, so ps_mu = E[x], ps_ms = E[x^2] directly
        nc.vector.tensor_copy(mu_all[:, bc, :], ps_mu)
        msq_t = mspool.tile([128, BCH], F32, name="msq_t", tag="msq_t")
        nc.vector.scalar_tensor_tensor(
            out=msq_t, in0=mu_all[:, bc, :], scalar=-1.0, in1=mu_all[:, bc, :],
            op0=ALU.mult, op1=ALU.mult)
        nc.vector.tensor_add(var_all[:, bc, :], ps_ms, msq_t)

    def emit_sqrt(j, var_all, mu_all):
        """One sqrt + reciprocal over the joint's full [128, nbc*BCH] var."""
        v_flat = var_all.rearrange("p i b -> p (i b)")
        std_all = stpool.tile([128, nbc, BCH], BF16, name="std_all", tag="std_all")
        s_flat = std_all.rearrange("p i b -> p (i b)")
        nc.scalar.activation(s_flat, v_flat, AF.Sqrt, bias=eps_t, scale=1.0)
        with nc.allow_low_precision(reason="bf16 rstd: 0.1% rms, tol 2e-2"):
            nc.vector.reciprocal(s_flat, s_flat)
        jst[j] = (mu_all, std_all)

    def emit_weights(j):
        w1_t = [wpool.tile([128, M], BF16, name=f"w1_{dc}", tag=f"w1_{dc}")
                for dc in range(NDC)]
        for dc in range(NDC):
            nc.sync.dma_start(out=w1_t[dc],
                              in_=w1[j, dc * 128:(dc + 1) * 128, :])
        w2_t = wpool.tile([128, NMC, 3], BF16, name="w2_t", tag="w2_t")
        nc.sync.dma_start(out=w2_t, in_=w2[j])
        b1_t = wpool.tile([128, NMC], F32, name="b1_t", tag="b1_t")
        nc.sync.dma_start(out=b1_t, in_=b1[j])
        b2r_t = wpool.tile([128, 1], F32, name="b2r_t", tag="b2r_t")
        nc.sync.dma_start(out=b2r_t, in_=b2[j])
        jw[j] = (w1_t, w2_t, b1_t, b2r_t)

    def emit_gemm_block(j, blk):
        """Gemms for a block of NGRP batch chunks of joint j.

        Loop order mc -> dc -> bc so one LDWEIGHTS serves NGRP matmuls;
        gemm2 for chunk mc is emitted after gemm1 of chunk mc+1 so it
        never waits on the trailing gelu (in-order PE).
        """
        w1_t, w2_t, b1_t, b2r_t = jw[j]
        mu_all, rstd_all = jst[j]
        bcs = [blk * ngrp + g for g in range(ngrp)]

        xns = []
        if const_xn:
            xns = list(cxn)
        for g, bc in enumerate(bcs):
            if const_xn:
                break
            bsl = slice(bc * BCH, (bc + 1) * BCH)
            xt2 = x2pool.tile([128, NDC, BCH], BF16, name="xt2", tag=f"xt2_{g}")
            nc.sync.dma_start(
                out=xt2,
                in_=xT[j, :, bsl].rearrange("(dc p) b -> p dc b", p=128),
            )
            if no_norm:  # perf probe: skip normalization (wrong results)
                xns.append(xt2)
                continue
            mu_b = _bcast_dc(mu_all[:, bc, :], NDC)
            rstd_b = _bcast_dc(rstd_all[:, bc, :], NDC)
            xn = xnpool.tile([128, NDC, BCH], BF16, name="xn", tag=f"xn{g}")
            nc.vector.tensor_sub(xn, xt2, mu_b)
            nc.vector.tensor_mul(xn, xn, rstd_b)
            xns.append(xn)

        if no_g2:
            ps_y = None
        elif serial_g2:
            ps_y = [ps_g2.tile([3, BCH], F32, name="ps_y", tag=f"ps_y{g}",
                               bufs=1)
                    for g in range(ngrp)]
        else:
            ps_y = ps_g2.tile([128, BCH], F32, name="ps_y", tag="ps_y")
        prev = None  # (h_ts, mc) whose gemm2 is pending
        nmc = NMC // 2 if half_g1 else NMC
        for mc in range(nmc):
            msl = slice(mc * 128, (mc + 1) * 128)
            ps_hs = [ps_g1.tile([128, BCH], F32, name="ps_h", tag=f"ps_h{g}",
                               bufs=ps_h_bufs)
                     for g in range(ngrp)]
            if g_outer:
                # per-group contraction chains: 4 consecutive matmuls into
                # the same PSUM bank (no bank cycling between matmuls)
                for g in range(ngrp):
                    for dc in range(NDC):
                        nc.tensor.matmul(ps_hs[g], w1_t[dc][:, msl],
                                         xns[g][:, dc, :],
                                         start=(dc == 0), stop=(dc == NDC - 1))
            else:
                for dc in range(NDC):
                    for g in range(ngrp):
                        nc.tensor.matmul(ps_hs[g], w1_t[dc][:, msl],
                                         xns[g][:, dc, :],
                                         start=(dc == 0), stop=(dc == NDC - 1))
            if prev is not None and not no_g2:
                emit_g2(ps_y, w2_t, *prev)
            h_ts = []
            for g in range(ngrp):
                h_t = hpool.tile([128, BCH], BF16, name="h_t", tag=f"h_t{g}")
                if half_gelu:  # perf probe: half the ACT work (wrong results)
                    nc.scalar.activation(h_t[:, 0:BCH // 2], ps_hs[g][:, 0:BCH // 2],
                                         AF.Gelu, bias=b1_t[:, mc:mc + 1], scale=1.0)
                else:
                    nc.scalar.activation(h_t, ps_hs[g],
                                         AF.Gelu if gelu else AF.Identity,
                                         bias=b1_t[:, mc:mc + 1], scale=1.0)
                h_ts.append(h_t)
            prev = (h_ts, mc)
        if not no_g2:
            emit_g2(ps_y, w2_t, *prev)

        y4 = None
        if not serial_g2:
            y4 = opool.tile([128, BCH], F32, name="y4", tag="y4")
        if no_g2:  # perf probe: no gemm2; emit dummy output
            nc.vector.memset(y4, 0.0)
            for g, bc in enumerate(bcs):
                bsl = slice(bc * BCH, (bc + 1) * BCH)
                nc.sync.dma_start(out=yT[j, :, bsl], in_=y4[0:3, :])
            return
        for g, bc in enumerate(bcs):
            psl = slice(32 * g, 32 * g + 3)
            bsl = slice(bc * BCH, (bc + 1) * BCH)
            if serial_g2:
                y4g = opool.tile([3, BCH], F32, name="y4g", tag="y4g", bufs=2)
                nc.vector.tensor_scalar_add(y4g, ps_y[g][:, :], b2r_t[0:3, :])
                nc.sync.dma_start(out=yT[j, :, bsl], in_=y4g)
            else:
                nc.vector.tensor_scalar_add(y4[psl, :], ps_y[psl, :],
                                            b2r_t[psl, :])
                nc.sync.dma_start(out=yT[j, :, bsl], in_=y4[psl, :])

    def emit_g2(ps_y, w2_t, h_ts, mc):
        for g in range(ngrp):
            if serial_g2:
                nc.tensor.matmul(ps_y[g][:, :], w2_t[:, mc, :],
                                 h_ts[g],
                                 start=(mc == 0), stop=(mc == NMC - 1),
                                 skip_group_check=True)
            else:
                nc.tensor.matmul(ps_y[32 * g:32 * g + 3, :], w2_t[:, mc, :],
                                 h_ts[g],
                                 start=(mc == 0),
                                 stop=(mc == (NMC // 2 if half_g1 else NMC) - 1),
                                 tile_position=(0, 32 * g),
                                 skip_group_check=True)

    # Phase 1: stats for all joints, sqrts back-to-back on ACT
    if no_phase1:  # perf probe: skip stats (wrong results)
        mu0 = stpool.tile([128, nbc, BCH], BF16, name="mu_all", tag="mu_all",
                          bufs=1)
        std0 = stpool.tile([128, nbc, BCH], BF16, name="std_all",
                           tag="std_all", bufs=1)
        nc.vector.memset(mu0, 0.0)
        nc.vector.memset(std0, 1.0)
        for j in range(jpc):
            jst[j] = (mu0, std0)
        emit_weights(0)
    else:
        for j in range(jpc):
            var_all = mspool.tile([128, nbc, BCH], BF16, name="var_all",
                                  tag="var_all")
            mu_all = stpool.tile([128, nbc, BCH], BF16, name="mu_all",
                                 tag="mu_all")
            for bc in range(nbc):
                emit_stats(j, bc, var_all, mu_all)
            emit_sqrt(j, var_all, mu_all)
            if j == 0:
                emit_weights(0)

    # Phase 2: gemms
    nblk = nbc // ngrp
    for j in range(jpc):
        if j + 1 < jpc:
            emit_weights(j + 1)
        for blk in range(nblk):
            emit_gemm_block(j, blk)


def _build_nc(jpc=JPC, nbc=NBC, reps=1, gelu=True, **kw):
    nc = bacc.Bacc("TRN2", target_bir_lowering=False, debug=False, num_devices=NCORES)
    with tile.TileContext(nc) as tc, ExitStack() as ctx:
        if reps == 1:
            build_body(nc, tc, ctx, jpc, nbc, gelu, **kw)
        else:
            # timing variant: repeat the whole body in a hardware loop
            def body(_i, unroll=1):
                with ExitStack() as c2:
                    build_body(nc, tc, c2, jpc, nbc, gelu, **kw)
            with tc.For_i(0, reps, 1) as i:
                body(i)
    nc.compile()
    return nc


def _pack_inputs(x, ln_g, ln_b, w1, b1, w2, b2):
    x = np.asarray(x)
    w1 = np.asarray(w1, dtype=np.float32)
    b1 = np.asarray(b1, dtype=np.float32)
    w2 = np.asarray(w2, dtype=np.float32)
    b2 = np.asarray(b2, dtype=np.float32)
    ln_g = np.asarray(ln_g)
    ln_b = np.asarray(ln_b)

    # fold LN affine into w1/b1: (xn*g + b) @ w1 == xn @ (g*w1) + b @ w1
    if not (ln_b == 0.0).all():
        b1 = b1 + np.einsum("jd,jdm->jm", ln_b, w1)
    if not (ln_g == 1.0).all():
        w1 = ln_g[:, :, None] * w1

    bf16 = ml_dtypes.bfloat16
    xT = np.ascontiguousarray(x.transpose(1, 2, 0)).astype(bf16)   # [J, D, B]
    w1p = w1.astype(bf16)
    w2p = np.ascontiguousarray(
        w2.reshape(J, NMC, 128, 3).transpose(0, 2, 1, 3)).astype(bf16)
    b1p = np.ascontiguousarray(
        b1.reshape(J, NMC, 128).transpose(0, 2, 1))                # [J, 128, NMC]
    # b2 replicated to partitions {0, 32, 64, 96} (gemm2 col-tile groups)
    b2p = np.zeros((J, 128, 1), dtype=np.float32)
    for g in range(NGRP):
        b2p[:, 32 * g:32 * g + 3, 0] = b2
    ones = np.full((128, 128), RD, dtype=bf16)  # 1/512 exact in bf16

    in_maps = []
    for c in range(NCORES):
        js = slice(c * JPC, (c + 1) * JPC)
        in_maps.append({
            "xT": xT[js],
            "w1": np.ascontiguousarray(w1p[js]),
            "b1": b1p[js],
            "w2": w2p[js],
            "b2": b2p[js],
            "ones": ones,
        })
    return in_maps


def kernel(x, ln_g, ln_b, w1, b1, w2, b2):
    if "nc" not in _CACHE:
        _CACHE["nc"] = _build_nc()
    nc = _CACHE["nc"]

    in_maps = _pack_inputs(x, ln_g, ln_b, w1, b1, w2, b2)
    res = bass_utils.run_bass_kernel_spmd(nc, in_maps, core_ids=list(range(NCORES)))

    # yT per core: [JPC, 3, B] -> y [B, 1, J, 3]
    yT = np.stack([res.results[c]["yT"] for c in range(NCORES)])  # [8, JPC, 3, B]
    y = yT.reshape(J, 3, B).transpose(2, 0, 1)[:, None, :, :]
    return np.ascontiguousarray(y.astype(np.float32))
